# revision 31
# baseline (speedup 1.0000x reference)
"""Multi-head attention layer (B=4, L=2048, D=1024, H=16) on 8 TRN2 NeuronCores.

Sharding: core c handles batch b = c//2 and heads [8*(c%2), 8*(c%2)+8) —
batch-parallel x tensor-parallel over heads.  Host sums the two partial
outputs per batch and adds bv@Wo + bo (bk drops exactly by softmax shift
invariance).

Per-core dataflow (all matmul inputs bf16, fp32 accumulation):
  qT/kT = Wq/Wk_slice as stationary against xT  -> [512, 2048] (e on partitions)
  v     = x @ Wv_slice (+ones col per head)      -> [2048, 8*65]
  scores transposed: ST[s, l] chunks = kT_h stationary vs qT_h
  A = exp(ST/8): ACT for most tiles, DVE Schraudolph-int16 bitcast for a few
  V[l, 65] = A_chunk.T @ v_aug accumulation (ones col -> softmax denom in
  col 64); normalize via batched reciprocal + tensor_scalar (DVE);
  PE-transpose V pairs into VT; out_partial = VT.T @ Wo_slice.

Scheduling: blocks are pair-major (m outer, lt inner).  Each block's g-loop
emits: 4 score matmuls, 2 exps, one AV slice (one (h2,j) 16-matmul chain) of
the PREVIOUS block, plus interleaved "extras" (qk/v projection quanta,
out-projection pieces) — so the PE queue always has score matmuls near the
front and the ACT exp pipeline never starves behind long matmul bursts.

Perf notes (HW-measured):
- V^T for the out-projection comes from DMA-XBAR transposes (sync queue for
  pairs 0-2, scalar queue for pair 3) instead of PE transposes + ACT copies.
- All psum->sbuf evacuations (projection quanta, out-proj pieces) are
  DEFERRED 1-2 g-slots behind their matmul chains: the ACT/DVE queues are
  strict FIFO, so an evac emitted right after its chain blocks the queue on
  the PE while the exps behind it starve the next block's score banks.
- 14 dummy warm-up matmuls on a zero tile run during the initial DMA wait so
  the PE HAM clock-gate (1.2 -> 2.4 GHz) fires before the first real block.
- qT/kT rotate through 2 tags (live range is 2 pairs), and normalized V is
  staged in one [128, 16*512] tile so each (h2) AV-norm is a single strided
  DVE op and the transposes slice it directly.
- Score K=64 row-tiled pairs issue concurrently but share moving-operand
  bandwidth (1 column-read/cycle): 64x64 quad tiling was measured to give
  zero net gain, so scores stay as plain row-tiled [64,128]x512 pairs.
"""

import sys
from contextlib import ExitStack

for _p in ("/opt/trn_rl_repo", "/root/.axon_site/_ro/trn_rl_repo"):
    if _p not in sys.path:
        sys.path.append(_p)

import numpy as np
import ml_dtypes

import concourse.bass as bass
import concourse.mybir as mybir
import concourse.tile as tile
from concourse import bacc
from concourse.bass_utils import run_bass_kernel_spmd
from concourse.masks import make_identity

BF16 = mybir.dt.bfloat16
F32 = mybir.dt.float32
AF = mybir.ActivationFunctionType

B, L, D = 4, 2048, 1024
N_CORES = 8
DH = 512          # per-core head dims (8 heads x 64)
E = 64
SCALE = 0.125     # 1/sqrt(E)

# Schraudolph fast-exp on DVE: i16 = trunc(A16*score + B16), bytes = bf16(exp)
# exp(s*x) = 2^(s*log2e*x); bf16 has 8exp+7mant like fp32.
LOG2E = 1.4426950408889634
EXP_C = -5.0      # sawtooth-centering correction (tuned in numerics.py)
EXP_A16 = 128.0 * SCALE * LOG2E
EXP_B16 = 16256.0 + EXP_C
# h2=0 exp tiles run whole ([128,1024]) on ACT.  h2=1 scores land in two
# single-bank psum tags and exp as [128,512] halves — mostly on DVE (658ns
# each, freeing the score bank earlier than a 1457ns full tile), with a few
# halves on ACT for load balance.  (g, c2) pairs listed here go to ACT.
ACT_H1_HALVES = {(0, 0), (2, 0), (4, 0)}

KD = D // 128     # 8 contraction chunks for projections
NL = L // 512     # 4 l-tiles
NS = L // 128     # 16 s-chunks / l-subs


def build_attention_nc():
    nc = bacc.Bacc("TRN2", target_bir_lowering=False, debug=False)

    xT_d = nc.dram_tensor("xT", [D, L], BF16, kind="ExternalInput").ap()
    wq_d = nc.dram_tensor("wq", [D, DH], BF16, kind="ExternalInput").ap()
    wk_d = nc.dram_tensor("wk", [D, DH], BF16, kind="ExternalInput").ap()
    wv_d = nc.dram_tensor("wv", [D, DH], BF16, kind="ExternalInput").ap()
    wo_d = nc.dram_tensor("wo", [DH, D], BF16, kind="ExternalInput").ap()
    bq_d = nc.dram_tensor("bq", [DH, 1], F32, kind="ExternalInput").ap()
    out_d = nc.dram_tensor("out", [L, D], BF16, kind="ExternalOutput").ap()

    with tile.TileContext(nc) as tc, ExitStack() as ctx:
        const_pool = ctx.enter_context(tc.tile_pool(name="const", bufs=1))
        w_pool = ctx.enter_context(tc.tile_pool(name="w", bufs=1))
        qk_pool = ctx.enter_context(tc.tile_pool(name="qk", bufs=1))
        v_pool = ctx.enter_context(tc.tile_pool(name="v", bufs=1))
        at_pool = ctx.enter_context(tc.tile_pool(name="at", bufs=16))
        vs_pool = ctx.enter_context(tc.tile_pool(name="vs", bufs=1))
        vt_pool = ctx.enter_context(tc.tile_pool(name="vt", bufs=1))
        rec_pool = ctx.enter_context(tc.tile_pool(name="rec", bufs=8))
        osb_pool = ctx.enter_context(tc.tile_pool(name="osb", bufs=2))

        st_ps = ctx.enter_context(tc.tile_pool(name="st_ps", bufs=1, space="PSUM"))
        av_ps = ctx.enter_context(tc.tile_pool(name="av_ps", bufs=1, space="PSUM"))
        tr_ps = ctx.enter_context(tc.tile_pool(name="tr_ps", bufs=1, space="PSUM"))
        out_ps = ctx.enter_context(tc.tile_pool(name="out_ps", bufs=1, space="PSUM"))
        phase1_ctx = ExitStack()
        xt_pool = phase1_ctx.enter_context(tc.tile_pool(name="xt", bufs=1))

        bq_sb = const_pool.tile([128, 4], F32, tag="bq", name="bq_sb")
        for m in range(4):
            nc.sync.dma_start(bq_sb[:, m : m + 1], bq_d[128 * m : 128 * m + 128, :])

        # PE warmup: dummy N=256 matmuls on a zero tile fill the initial x/W
        # DMA wait (PE queue is FIFO, so they run before any real matmul).
        # The HAM activity window fires ~3.4us in, so the first real block
        # runs at 2.4 GHz instead of spending its first ~14us cold at 1.2.
        warm = const_pool.tile([128, 512], BF16, tag="warm", name="warm")
        nc.vector.memset(warm[:], 0.0)
        for w_i in range(14):
            wp = st_ps.tile([128, 256], F32, tag="st0", name="warmps")
            nc.tensor.matmul(
                wp[:], warm[:, 0:128], warm[:, 0:256], start=True, stop=True)

        xt = []
        for i in range(KD):
            t = xt_pool.tile([128, L], BF16, tag=f"xt{i}", name=f"xt{i}")
            xt.append(t)
        wq, wk, wv = [], [], []
        for name, lst in (("wq", wq), ("wk", wk), ("wv", wv)):
            for i in range(KD):
                t = xt_pool.tile([128, DH], BF16, tag=f"{name}{i}", name=f"{name}{i}")
                lst.append(t)
        # DMA order: interleave x/Wq/Wk chunks so the pair-0 projection
        # accumulation chains start as soon as the first chunks land, then Wv.
        # xt is split into column quarters: the first k/q projection groups
        # (n=0) only need cols 0-512 of every chunk, so the first matmul can
        # start after ~1/4 of the x traffic.
        for i in range(KD):
            nc.sync.dma_start(xt[i][:, 0:512], xT_d[128 * i : 128 * i + 128, 0:512])
            nc.sync.dma_start(wk[i][:], wk_d[128 * i : 128 * i + 128, :])
        for i in range(KD):
            nc.sync.dma_start(wq[i][:], wq_d[128 * i : 128 * i + 128, :])
        for i in range(KD):
            nc.sync.dma_start(xt[i][:, 512:1024], xT_d[128 * i : 128 * i + 128, 512:1024])
        for i in range(KD):
            nc.sync.dma_start(wv[i][:], wv_d[128 * i : 128 * i + 128, :])
        for q in range(2, 4):
            for i in range(KD):
                nc.sync.dma_start(xt[i][:, 512 * q : 512 * q + 512],
                                  xT_d[128 * i : 128 * i + 128, 512 * q : 512 * q + 512])
        wo = []
        for p in range(DH // 128):
            t = w_pool.tile([128, D], BF16, tag=f"wo{p}", name=f"wo{p}")
            nc.sync.dma_start(t[:], wo_d[128 * p : 128 * p + 128, :])
            wo.append(t)

        # qT/kT live for 2 pairs only (proj fill during pair m-1, scores in
        # pair m) -> rotate through 2 tags (m%2) to halve the SBUF footprint
        qT_, kT_ = {}, {}
        for mm2 in range(2):
            qT_[mm2] = qk_pool.tile([128, L], BF16, tag=f"qT{mm2}", name=f"qT{mm2}")
            kT_[mm2] = qk_pool.tile([128, L], BF16, tag=f"kT{mm2}", name=f"kT{mm2}")
        qT = [qT_[m % 2] for m in range(4)]
        kT = [kT_[m % 2] for m in range(4)]

        # projection evacuations are deferred one g-slot behind their matmul
        # chains: an evac emitted right after its chain blocks the strict-FIFO
        # DVE queue on the PE, stalling the exps queued behind it.
        proj_ps = {}

        def emit_qk_mms(m, which, n, tag):
            # one psum group (8 matmuls) of the q or k projection
            if tag in ("outp",):
                ps = out_ps.tile([128, 512], F32, tag=tag, name="proj")
            elif tag in ("tr",):
                ps = tr_ps.tile([128, 512], F32, tag=tag, name="proj")
            elif tag.startswith("av"):
                ps = av_ps.tile([128, 512], F32, tag=tag, name="proj")
            else:
                ps = st_ps.tile([128, 512], F32, tag=tag, name="proj")
            w_ = wq if which == "q" else wk
            for kd in range(KD):
                nc.tensor.matmul(
                    ps[:], w_[kd][:, 128 * m : 128 * m + 128],
                    xt[kd][:, 512 * n : 512 * n + 512],
                    start=(kd == 0), stop=(kd == KD - 1))
            proj_ps[(which, m, n)] = ps

        def emit_qk_evac(m, which, n):
            ps = proj_ps.pop((which, m, n))
            with tc.high_priority():
                if which == "q":
                    nc.vector.tensor_scalar_add(
                        qT[m][:, 512 * n : 512 * n + 512], ps[:], bq_sb[:, m : m + 1])
                else:
                    nc.vector.tensor_copy(kT[m][:, 512 * n : 512 * n + 512], ps[:])

        def emit_qk_proj(m, which, n, tag):
            emit_qk_mms(m, which, n, tag)
            emit_qk_evac(m, which, n)

        v_aug = [None] * NS

        def emit_v_mms(s, tag):
            pool = av_ps if tag.startswith("av") else out_ps if tag == "outp" else tr_ps
            ps = pool.tile([128, 512], F32, tag=tag, name="proj")
            for kd in range(KD):
                nc.tensor.matmul(
                    ps[:], xt[kd][:, 128 * s : 128 * s + 128], wv[kd][:],
                    start=(kd == 0), stop=(kd == KD - 1))
            proj_ps[("v", s)] = ps

        def emit_v_evac(s):
            ps = proj_ps.pop(("v", s))
            t = v_pool.tile([128, 520], BF16, tag=f"v{s}", name=f"vaug{s}")
            t3 = t[:].rearrange("p (h e) -> p h e", h=8)
            with tc.high_priority():
                nc.vector.tensor_copy(t3[:, :, 0:64], ps[:].rearrange("p (h e) -> p h e", h=8))
                nc.vector.memset(t3[:, :, 64:65], 1.0)
            v_aug[s] = t

        # one [128, 16*512] staging tile for normalized V: lets the AV norm
        # write all 4 j-destinations of an (h2) group in ONE strided DVE op,
        # and the out-projection transposes slice it directly.
        vs_all = vs_pool.tile([128, NS * DH], BF16, tag="vs", name="vs_all")
        vs3 = vs_all[:].rearrange("p (ls c) -> p ls c", ls=NS)

        # V^T tiles for the out-projection, produced by DMA-XBAR transposes
        # (SBUF->SBUF, sync queue) instead of PE transposes + ACT copies.
        vt_t = [[vt_pool.tile([128, 128], BF16, tag=f"vt{ls}_{p}", name=f"vt{ls}_{p}")
                 for p in range(4)] for ls in range(NS)]

        def emit_vt(ls, p, eng=None):
            # p<3 transposes ride the sync queue (slack mid-pairs); pair-3 and
            # final-loop ones go via the scalar queue so they are not FIFO'd
            # behind out-DMAs + semaphore waits on sync (measured 2.8us stalls)
            (eng or nc.sync).dma_start_transpose(
                vt_t[ls][p][:], vs_all[:, DH * ls + 128 * p : DH * ls + 128 * p + 128])

        def emit_av_slice(prev, g, perj=False):
            # one (h2, j) 16-matmul AV chain of the previous block
            m, lt, ats, avs = prev
            h2, j = g % 2, g // 2
            if h2 not in avs:
                avs[h2] = av_ps.tile([128, 260], F32, tag=f"av{h2}", name=f"av{h2}")
            avp = avs[h2]
            for s in range(NS):
                gg, c2 = divmod(s, 2)
                nc.tensor.matmul(
                    avp[:, 65 * j : 65 * j + 65],
                    ats[h2][gg][:, 512 * c2 + 128 * j : 512 * c2 + 128 * j + 128],
                    v_aug[s][:, 65 * (2 * m + h2) : 65 * (2 * m + h2) + 65],
                    start=(s == 0), stop=(s == NS - 1))
            if perj:
                emit_av_norm(prev, h2, j)
            elif j == 3:
                emit_av_norm(prev, h2)

        def emit_av_norm(prev, h2, j=None):
            m, lt, ats, avs = prev
            h = 2 * m + h2
            avp = avs[h2]
            with tc.high_priority():
                r = rec_pool.tile([128, 4], F32, tag="rec", name="rec")
                if j is None:
                    # batched: one reciprocal + one strided tensor_tensor for
                    # all 4 j-destinations (r broadcast along e via stride-0)
                    nc.vector.reciprocal(r[:], avp[:, 64:260:65])
                    src = avp[:].rearrange("p (j e) -> p j e", j=4)[:, :, 0:64]
                    dst = vs3[:, 4 * lt : 4 * lt + 4, 64 * h : 64 * h + 64]
                    rb = r[:].rearrange("p (j o) -> p j o", j=4).broadcast_to(
                        (128, 4, 64))
                    nc.vector.tensor_tensor(dst, src, rb, mybir.AluOpType.mult)
                else:
                    nc.vector.reciprocal(
                        r[:, j : j + 1], avp[:, 65 * j + 64 : 65 * j + 65])
                    nc.vector.tensor_scalar_mul(
                        vs3[:, 4 * lt + j, 64 * h : 64 * h + 64],
                        avp[:, 65 * j : 65 * j + 64], r[:, j : j + 1])

        # out-projection pieces are split into an MM part and a DEFERRED
        # evacuation part (2 g-slots later): a piece's psum->sbuf copies used
        # to sit in the strict-FIFO ACT/DVE queues blocking 1-3us on the MM
        # chain while the PE stalled on the exps queued behind them.
        pending_ops = {}

        def emit_piece_mms(ls):
            ops = []
            for d2 in range(2):
                # alternate psum tags so the d2=1 chain does not wait for the
                # d2=0 evacuation (tr is free of transposes now)
                pool = out_ps if d2 == 0 else tr_ps
                op = pool.tile([128, 512], F32, tag=("outp" if d2 == 0 else "tr"),
                               name="outp")
                for p in range(4):
                    nc.tensor.matmul(
                        op[:], vt_t[ls][p][:], wo[p][:, 512 * d2 : 512 * d2 + 512],
                        start=(p == 0), stop=(p == 3))
                ops.append(op)
            pending_ops[ls] = ops

        def emit_piece_evac(ls):
            ops = pending_ops.pop(ls)
            osb = osb_pool.tile([128, D], BF16, tag="osb", name="osb")
            for d2 in range(2):
                dst = osb[:, 512 * d2 : 512 * d2 + 512]
                if d2 == 0:
                    nc.scalar.copy(dst, ops[d2][:])
                else:
                    nc.vector.tensor_copy(dst, ops[d2][:])
                nc.sync.dma_start(
                    out_d[128 * ls : 128 * ls + 128, 512 * d2 : 512 * d2 + 512],
                    dst)

        def emit_block(m, lt, prev, extras, perj=False):
            ats = {0: [], 1: []}
            for g in range(8):
                st0 = st_ps.tile([128, 1024], F32, tag="st0", name="st0")
                st1 = {c2: st_ps.tile([128, 512], F32, tag=f"st1{c2}", name=f"st1{c2}")
                       for c2 in range(2)}
                # interleave row groups (h=0, h=1, h=0, h=1): each MM's
                # LDWEIGHTS targets the opposite array half of the in-flight
                # matmul, so the per-subarray pull-ahead hides it (emitting
                # both c2 of one half back-to-back exposes ~107ns per pair)
                for c2 in range(2):
                    s = 2 * g + c2
                    nc.tensor.matmul(
                        st0[:, 512 * c2 : 512 * c2 + 512],
                        kT[m][0:64, 128 * s : 128 * s + 128],
                        qT[m][0:64, 512 * lt : 512 * lt + 512],
                        start=True, stop=True)
                    nc.tensor.matmul(
                        st1[c2][:],
                        kT[m][64:128, 128 * s : 128 * s + 128],
                        qT[m][64:128, 512 * lt : 512 * lt + 512],
                        start=True, stop=True)
                at0 = at_pool.tile([128, 1024], BF16, tag="at0", name="at0")
                nc.scalar.activation(at0[:], st0[:], AF.Exp, scale=SCALE)
                ats[0].append(at0)
                at1 = at_pool.tile([128, 1024], BF16, tag="at1", name="at1")
                for c2 in range(2):
                    dst = at1[:, 512 * c2 : 512 * c2 + 512]
                    # pair 3: ACT also carries the piece-evac copies, so all
                    # h1-halves go to the DVE Schraudolph path there
                    if m < 3 and (g, c2) in ACT_H1_HALVES:
                        nc.scalar.activation(dst, st1[c2][:], AF.Exp, scale=SCALE)
                    else:
                        nc.vector.tensor_scalar(
                            dst.bitcast(mybir.dt.int16), st1[c2][:],
                            EXP_A16, EXP_B16,
                            mybir.AluOpType.mult, mybir.AluOpType.add)
                ats[1].append(at1)
                if prev is not None:
                    emit_av_slice(prev, g, perj=perj)
                for fn in extras.get(g, ()):
                    fn()
            return ats

        # ---- prologue: only q(0,0) + k(0,0); the rest streams into blocks ----
        emit_qk_proj(0, "q", 0, "st0")
        emit_qk_proj(0, "k", 0, "st10")

        rot = ["outp", "tr"]
        vrot = ["av0", "av1", "outp", "tr"]

        prev = None
        pq_pair = []
        carry_evac = []
        for m in range(4):
            # q(m,2)/q(m,3) quanta left over from pair m-1's blocks land in
            # block (m,0): it otherwise has the least PE work per g (no
            # extras), which is exactly where HAM re-throttles the PE clock.
            leftover = pq_pair
            if m < 3:
                # pair m+1 projection quanta, ordered by score-consumption
                # deadline in block (m+1, 0): k first (all s needed), q n=0
                # early (lt=0), q n=1..3 late (lt>=1 blocks).
                nx = m + 1
                pq_pair = [(nx, "k", 0), (nx, "q", 0), (nx, "k", 1),
                           (nx, "k", 2), (nx, "k", 3), (nx, "q", 1),
                           (nx, "q", 2), (nx, "q", 3)]
            for lt in range(NL):
                extras = {}
                if m == 0 and lt == 0:
                    # stream remaining pair-0 k (needed at g=2n) and q, plus
                    # the 16 v-projection groups, into the first block;
                    # evacuations go one slot behind their chains
                    for i, n in enumerate((1, 2, 3)):
                        extras.setdefault(2 * i, []).append(
                            lambda n=n: emit_qk_mms(0, "k", n, rot[n % 2]))
                        extras.setdefault(2 * i + 1, []).append(
                            lambda n=n: emit_qk_evac(0, "k", n))
                        extras.setdefault(2 * i + 1, []).append(
                            lambda n=n: emit_qk_mms(0, "q", n, rot[(n + 1) % 2]))
                        extras.setdefault(2 * i + 2, []).append(
                            lambda n=n: emit_qk_evac(0, "q", n))
                    for s2 in range(NS):
                        extras.setdefault(s2 // 2, []).append(
                            lambda s2=s2: emit_v_mms(s2, vrot[s2 % 4]))
                        extras.setdefault(min(s2 // 2 + 1, 7), []).append(
                            lambda s2=s2: emit_v_evac(s2))
                elif m < 3 and lt >= 1:
                    # 3/3/0 projection quanta in blocks (m, 1..2); the last
                    # two quanta carry over to block (m+1, 0) as leftovers
                    for i in range(3 if lt < 3 else 0):
                        if pq_pair:
                            pm, w, n = pq_pair.pop(0)
                            extras.setdefault(2 * i + 2, []).append(
                                lambda pm=pm, w=w, n=n, i=i:
                                    emit_qk_mms(pm, w, n, rot[i % 2]))
                            extras.setdefault(2 * i + 3, []).append(
                                lambda pm=pm, w=w, n=n: emit_qk_evac(pm, w, n))
                if lt == 0 and leftover:
                    for i, (pm, w, n) in enumerate(leftover):
                        extras.setdefault(2 * i + 2, []).append(
                            lambda pm=pm, w=w, n=n, i=i:
                                emit_qk_mms(pm, w, n, rot[i % 2]))
                        extras.setdefault(2 * i + 3, []).append(
                            lambda pm=pm, w=w, n=n: emit_qk_evac(pm, w, n))
                    leftover = []
                perj = m == 3 and lt > 0
                if perj:
                    # prev AV (pair 3, per-j normalized) feeds the lt-1
                    # out-projection pieces: norm-j lands at slice g=2j+1 (so
                    # the p=3 transpose DMA goes there), piece j at g=2j+2,
                    # its evacuation deferred to g=2j+4 (or the next block).
                    for i, ls_c in enumerate(carry_evac):
                        extras.setdefault(2 * i, []).append(
                            lambda ls_c=ls_c: emit_piece_evac(ls_c))
                    del carry_evac[:]
                    for j in range(4):
                        ls = 4 * (lt - 1) + j
                        extras.setdefault(min(2 * j + 1, 7), []).append(
                            lambda ls=ls: emit_vt(ls, 3, nc.scalar))
                        extras.setdefault(min(2 * j + 2, 7), []).append(
                            lambda ls=ls: emit_piece_mms(ls))
                        if 2 * j + 4 <= 7:
                            extras.setdefault(2 * j + 4, []).append(
                                lambda ls=ls: emit_piece_evac(ls))
                        else:
                            carry_evac.append(ls)
                ats = emit_block(m, lt, prev, extras, perj=perj)
                if prev is not None and prev[0] < 3:
                    # prev block's batched norm just landed inside this block:
                    # launch its 4 V^T transpose DMAs (dep-ordered sync queue)
                    for j in range(4):
                        emit_vt(4 * prev[1] + j, prev[0])
                prev = (m, lt, ats, {})
        # final block's AV (per-j normalized) interleaved with the last
        # out-projection pieces so the tail pipeline stays short
        for g in range(8):
            if g in (0, 2) and carry_evac:
                emit_piece_evac(carry_evac.pop(0))
            emit_av_slice(prev, g, perj=True)
            if g % 2 == 1:
                emit_vt(12 + g // 2, 3, nc.scalar)
                emit_piece_mms(12 + g // 2)
            if g == 5:
                emit_piece_evac(12)
            if g == 7:
                emit_piece_evac(13)
        emit_piece_evac(14)
        emit_piece_evac(15)
        phase1_ctx.close()

    nc.compile()
    return nc


_NC_CACHE = []


def _make_in_maps(inputs):
    x = np.asarray(inputs["x"], dtype=np.float32)
    Wq = np.asarray(inputs["Wq"], dtype=np.float32)
    Wk = np.asarray(inputs["Wk"], dtype=np.float32)
    Wv = np.asarray(inputs["Wv"], dtype=np.float32)
    Wo = np.asarray(inputs["Wo"], dtype=np.float32)
    bq = np.asarray(inputs["bq"], dtype=np.float32)
    bf = ml_dtypes.bfloat16
    in_maps = []
    for c in range(N_CORES):
        b, hh = divmod(c, 2)
        sl = slice(DH * hh, DH * hh + DH)
        in_maps.append({
            "xT": np.ascontiguousarray(x[b].T).astype(bf),
            "wq": np.ascontiguousarray(Wq[:, sl]).astype(bf),
            "wk": np.ascontiguousarray(Wk[:, sl]).astype(bf),
            "wv": np.ascontiguousarray(Wv[:, sl]).astype(bf),
            "wo": np.ascontiguousarray(Wo[sl, :]).astype(bf),
            "bq": np.ascontiguousarray(bq[sl]).reshape(DH, 1).astype(np.float32),
        })
    return in_maps


def kernel(x, Wq, bq, Wk, bk, Wv, bv, Wo, bo):
    x = np.asarray(x, dtype=np.float32)
    Wq = np.asarray(Wq, dtype=np.float32)
    Wk = np.asarray(Wk, dtype=np.float32)
    Wv = np.asarray(Wv, dtype=np.float32)
    Wo = np.asarray(Wo, dtype=np.float32)
    bq = np.asarray(bq, dtype=np.float32)
    bv = np.asarray(bv, dtype=np.float32)
    bo = np.asarray(bo, dtype=np.float32)

    if not _NC_CACHE:
        _NC_CACHE.append(build_attention_nc())
    nc = _NC_CACHE[0]

    in_maps = _make_in_maps(dict(x=x, Wq=Wq, bq=bq, Wk=Wk, Wv=Wv, Wo=Wo))

    res = run_bass_kernel_spmd(nc, in_maps, list(range(N_CORES)))
    parts = [np.asarray(res.results[c]["out"]).astype(np.float32)
             for c in range(N_CORES)]
    out = np.stack([parts[2 * b] + parts[2 * b + 1] for b in range(B)])
    out += (bv @ Wo + bo)[None, None, :]
    return out.astype(np.float32)



# revision 33
# speedup vs baseline: 1.1743x; 1.1743x over previous
"""Multi-head attention layer (B=4, L=2048, D=1024, H=16) on 8 TRN2 NeuronCores.

Sharding: core c handles batch b = c//2 and heads [8*(c%2), 8*(c%2)+8) —
batch-parallel x tensor-parallel over heads.  Host sums the two partial
outputs per batch and adds bv@Wo + bo (bk drops exactly by softmax shift
invariance).

Per-core dataflow (all matmul inputs bf16, fp32 accumulation):
  qT/kT = Wq/Wk_slice as stationary against xT  -> [512, 2048] (e on partitions)
  v     = x @ Wv_slice (+ones col per head)      -> [2048, 8*65]
  scores transposed: ST[s, l] chunks = kT_h stationary vs qT_h
  A = exp(ST/8): ACT for most tiles, DVE Schraudolph-int16 bitcast for a few
  V[l, 65] = A_chunk.T @ v_aug accumulation (ones col -> softmax denom in
  col 64); normalize via batched reciprocal + tensor_scalar (DVE);
  PE-transpose V pairs into VT; out_partial = VT.T @ Wo_slice.

Scheduling: blocks are pair-major (m outer, lt inner).  Each block's g-loop
emits: 4 score matmuls, 2 exps, one AV slice (one (h2,j) 16-matmul chain) of
the PREVIOUS block, plus interleaved "extras" (qk/v projection quanta,
out-projection pieces) — so the PE queue always has score matmuls near the
front and the ACT exp pipeline never starves behind long matmul bursts.

Perf notes (HW-measured):
- V^T for the out-projection comes from DMA-XBAR transposes (sync queue for
  pairs 0-2, scalar queue for pair 3) instead of PE transposes + ACT copies.
- All psum->sbuf evacuations (projection quanta, out-proj pieces) are
  DEFERRED 1-2 g-slots behind their matmul chains: the ACT/DVE queues are
  strict FIFO, so an evac emitted right after its chain blocks the queue on
  the PE while the exps behind it starve the next block's score banks.
- 14 dummy warm-up matmuls on a zero tile run during the initial DMA wait so
  the PE HAM clock-gate (1.2 -> 2.4 GHz) fires before the first real block.
- qT/kT rotate through 2 tags (live range is 2 pairs), and normalized V is
  staged in one [128, 16*512] tile so each (h2) AV-norm is a single strided
  DVE op and the transposes slice it directly.
- Score K=64 row-tiled pairs issue concurrently but share moving-operand
  bandwidth (1 column-read/cycle): 64x64 quad tiling was measured to give
  zero net gain, so scores stay as plain row-tiled [64,128]x512 pairs.
"""

import sys
from contextlib import ExitStack

for _p in ("/opt/trn_rl_repo", "/root/.axon_site/_ro/trn_rl_repo"):
    if _p not in sys.path:
        sys.path.append(_p)

import numpy as np
import ml_dtypes

import concourse.bass as bass
import concourse.mybir as mybir
import concourse.tile as tile
from concourse import bacc
from concourse.bass_utils import run_bass_kernel_spmd
from concourse.masks import make_identity

BF16 = mybir.dt.bfloat16
F32 = mybir.dt.float32
AF = mybir.ActivationFunctionType

B, L, D = 4, 2048, 1024
N_CORES = 8
DH = 512          # per-core head dims (8 heads x 64)
E = 64
SCALE = 0.125     # 1/sqrt(E)

# Schraudolph fast-exp on DVE: i16 = trunc(A16*score + B16), bytes = bf16(exp)
# exp(s*x) = 2^(s*log2e*x); bf16 has 8exp+7mant like fp32.
LOG2E = 1.4426950408889634
EXP_C = -5.0      # sawtooth-centering correction (tuned in numerics.py)
EXP_A16 = 128.0 * SCALE * LOG2E
EXP_B16 = 16256.0 + EXP_C
# h2=0 exp tiles run whole ([128,1024]) on ACT.  h2=1 scores land in two
# single-bank psum tags and exp as [128,512] halves — mostly on DVE (658ns
# each, freeing the score bank earlier than a 1457ns full tile), with a few
# halves on ACT for load balance.  (g, c2) pairs listed here go to ACT.
ACT_H1_HALVES = {(0, 0), (2, 0), (4, 0)}

KD = D // 128     # 8 contraction chunks for projections
NL = L // 512     # 4 l-tiles
NS = L // 128     # 16 s-chunks / l-subs


def build_attention_nc():
    nc = bacc.Bacc("TRN2", target_bir_lowering=False, debug=False)

    xT_d = nc.dram_tensor("xT", [D, L], BF16, kind="ExternalInput").ap()
    wq_d = nc.dram_tensor("wq", [D, DH], BF16, kind="ExternalInput").ap()
    wk_d = nc.dram_tensor("wk", [D, DH], BF16, kind="ExternalInput").ap()
    wv_d = nc.dram_tensor("wv", [D, DH], BF16, kind="ExternalInput").ap()
    wo_d = nc.dram_tensor("wo", [DH, D], BF16, kind="ExternalInput").ap()
    bq_d = nc.dram_tensor("bq", [DH, 1], F32, kind="ExternalInput").ap()
    out_d = nc.dram_tensor("out", [L, D], BF16, kind="ExternalOutput").ap()

    with tile.TileContext(nc) as tc, ExitStack() as ctx:
        const_pool = ctx.enter_context(tc.tile_pool(name="const", bufs=1))
        w_pool = ctx.enter_context(tc.tile_pool(name="w", bufs=1))
        qk_pool = ctx.enter_context(tc.tile_pool(name="qk", bufs=1))
        v_pool = ctx.enter_context(tc.tile_pool(name="v", bufs=1))
        at_pool = ctx.enter_context(tc.tile_pool(name="at", bufs=16))
        vs_pool = ctx.enter_context(tc.tile_pool(name="vs", bufs=1))
        vt_pool = ctx.enter_context(tc.tile_pool(name="vt", bufs=1))
        rec_pool = ctx.enter_context(tc.tile_pool(name="rec", bufs=8))
        osb_pool = ctx.enter_context(tc.tile_pool(name="osb", bufs=2))

        st_ps = ctx.enter_context(tc.tile_pool(name="st_ps", bufs=1, space="PSUM"))
        av_ps = ctx.enter_context(tc.tile_pool(name="av_ps", bufs=1, space="PSUM"))
        tr_ps = ctx.enter_context(tc.tile_pool(name="tr_ps", bufs=1, space="PSUM"))
        out_ps = ctx.enter_context(tc.tile_pool(name="out_ps", bufs=1, space="PSUM"))
        phase1_ctx = ExitStack()
        xt_pool = phase1_ctx.enter_context(tc.tile_pool(name="xt", bufs=1))

        bq_sb = const_pool.tile([128, 4], F32, tag="bq", name="bq_sb")
        for m in range(4):
            nc.sync.dma_start(bq_sb[:, m : m + 1], bq_d[128 * m : 128 * m + 128, :])

        # PE warmup: dummy N=256 matmuls on a zero tile fill the initial x/W
        # DMA wait (PE queue is FIFO, so they run before any real matmul).
        # The HAM activity window fires ~3.4us in, so the first real block
        # runs at 2.4 GHz instead of spending its first ~14us cold at 1.2.
        warm = const_pool.tile([128, 512], BF16, tag="warm", name="warm")
        nc.vector.memset(warm[:], 0.0)
        for w_i in range(14):
            wp = st_ps.tile([128, 256], F32, tag="st0", name="warmps")
            nc.tensor.matmul(
                wp[:], warm[:, 0:128], warm[:, 0:256], start=True, stop=True)

        xt = []
        for i in range(KD):
            t = xt_pool.tile([128, L], BF16, tag=f"xt{i}", name=f"xt{i}")
            xt.append(t)
        wq, wk, wv = [], [], []
        for name, lst in (("wq", wq), ("wk", wk), ("wv", wv)):
            for i in range(KD):
                t = xt_pool.tile([128, DH], BF16, tag=f"{name}{i}", name=f"{name}{i}")
                lst.append(t)
        # DMA order: interleave x/Wq/Wk chunks so the pair-0 projection
        # accumulation chains start as soon as the first chunks land, then Wv.
        # xt is split into column quarters: the first k/q projection groups
        # (n=0) only need cols 0-512 of every chunk, so the first matmul can
        # start after ~1/4 of the x traffic.
        for i in range(KD):
            nc.sync.dma_start(xt[i][:, 0:512], xT_d[128 * i : 128 * i + 128, 0:512])
            nc.sync.dma_start(wk[i][:], wk_d[128 * i : 128 * i + 128, :])
        for i in range(KD):
            nc.sync.dma_start(wq[i][:], wq_d[128 * i : 128 * i + 128, :])
        for i in range(KD):
            nc.sync.dma_start(xt[i][:, 512:1024], xT_d[128 * i : 128 * i + 128, 512:1024])
        for i in range(KD):
            nc.sync.dma_start(wv[i][:], wv_d[128 * i : 128 * i + 128, :])
        for q in range(2, 4):
            for i in range(KD):
                nc.sync.dma_start(xt[i][:, 512 * q : 512 * q + 512],
                                  xT_d[128 * i : 128 * i + 128, 512 * q : 512 * q + 512])
        wo = []
        for p in range(DH // 128):
            t = w_pool.tile([128, D], BF16, tag=f"wo{p}", name=f"wo{p}")
            nc.sync.dma_start(t[:], wo_d[128 * p : 128 * p + 128, :])
            wo.append(t)

        # qT/kT live for 2 pairs only (proj fill during pair m-1, scores in
        # pair m) -> rotate through 2 tags (m%2) to halve the SBUF footprint
        qT_, kT_ = {}, {}
        for mm2 in range(2):
            qT_[mm2] = qk_pool.tile([128, L], BF16, tag=f"qT{mm2}", name=f"qT{mm2}")
            kT_[mm2] = qk_pool.tile([128, L], BF16, tag=f"kT{mm2}", name=f"kT{mm2}")
        qT = [qT_[m % 2] for m in range(4)]
        kT = [kT_[m % 2] for m in range(4)]

        # projection evacuations are deferred one g-slot behind their matmul
        # chains: an evac emitted right after its chain blocks the strict-FIFO
        # DVE queue on the PE, stalling the exps queued behind it.
        proj_ps = {}

        def emit_qk_mms(m, which, n, tag):
            # one psum group (8 matmuls) of the q or k projection
            if tag in ("outp",):
                ps = out_ps.tile([128, 512], F32, tag=tag, name="proj")
            elif tag in ("tr",):
                ps = tr_ps.tile([128, 512], F32, tag=tag, name="proj")
            elif tag.startswith("av"):
                ps = av_ps.tile([128, 512], F32, tag=tag, name="proj")
            else:
                ps = st_ps.tile([128, 512], F32, tag=tag, name="proj")
            w_ = wq if which == "q" else wk
            for kd in range(KD):
                nc.tensor.matmul(
                    ps[:], w_[kd][:, 128 * m : 128 * m + 128],
                    xt[kd][:, 512 * n : 512 * n + 512],
                    start=(kd == 0), stop=(kd == KD - 1))
            proj_ps[(which, m, n)] = ps

        def emit_qk_evac(m, which, n):
            ps = proj_ps.pop((which, m, n))
            with tc.high_priority():
                if which == "q":
                    nc.vector.tensor_scalar_add(
                        qT[m][:, 512 * n : 512 * n + 512], ps[:], bq_sb[:, m : m + 1])
                else:
                    nc.vector.tensor_copy(kT[m][:, 512 * n : 512 * n + 512], ps[:])

        def emit_qk_proj(m, which, n, tag):
            emit_qk_mms(m, which, n, tag)
            emit_qk_evac(m, which, n)

        v_aug = [None] * NS

        def emit_v_mms(s, tag):
            pool = av_ps if tag.startswith("av") else out_ps if tag == "outp" else tr_ps
            ps = pool.tile([128, 512], F32, tag=tag, name="proj")
            for kd in range(KD):
                nc.tensor.matmul(
                    ps[:], xt[kd][:, 128 * s : 128 * s + 128], wv[kd][:],
                    start=(kd == 0), stop=(kd == KD - 1))
            proj_ps[("v", s)] = ps

        def emit_v_evac(s):
            ps = proj_ps.pop(("v", s))
            t = v_pool.tile([128, 520], BF16, tag=f"v{s}", name=f"vaug{s}")
            t3 = t[:].rearrange("p (h e) -> p h e", h=8)
            with tc.high_priority():
                nc.vector.tensor_copy(t3[:, :, 0:64], ps[:].rearrange("p (h e) -> p h e", h=8))
                nc.vector.memset(t3[:, :, 64:65], 1.0)
            v_aug[s] = t

        # one [128, 16*512] staging tile for normalized V: lets the AV norm
        # write all 4 j-destinations of an (h2) group in ONE strided DVE op,
        # and the out-projection transposes slice it directly.
        vs_all = vs_pool.tile([128, NS * DH], BF16, tag="vs", name="vs_all")
        vs3 = vs_all[:].rearrange("p (ls c) -> p ls c", ls=NS)

        # V^T tiles for the out-projection, produced by DMA-XBAR transposes
        # (SBUF->SBUF, sync queue) instead of PE transposes + ACT copies.
        vt_t = [[vt_pool.tile([128, 128], BF16, tag=f"vt{ls}_{p}", name=f"vt{ls}_{p}")
                 for p in range(4)] for ls in range(NS)]

        def emit_vt(ls, p, eng=None):
            # p<3 transposes ride the sync queue (slack mid-pairs); pair-3 and
            # final-loop ones go via the scalar queue so they are not FIFO'd
            # behind out-DMAs + semaphore waits on sync (measured 2.8us stalls)
            (eng or nc.sync).dma_start_transpose(
                vt_t[ls][p][:], vs_all[:, DH * ls + 128 * p : DH * ls + 128 * p + 128])

        def emit_av_slice(prev, g, perj=False):
            # one (h2, j) 16-matmul AV chain of the previous block
            m, lt, ats, avs = prev
            h2, j = g % 2, g // 2
            if h2 not in avs:
                avs[h2] = av_ps.tile([128, 260], F32, tag=f"av{h2}", name=f"av{h2}")
            avp = avs[h2]
            for s in range(NS):
                gg, c2 = divmod(s, 2)
                nc.tensor.matmul(
                    avp[:, 65 * j : 65 * j + 65],
                    ats[h2][gg][:, 512 * c2 + 128 * j : 512 * c2 + 128 * j + 128],
                    v_aug[s][:, 65 * (2 * m + h2) : 65 * (2 * m + h2) + 65],
                    start=(s == 0), stop=(s == NS - 1))
            if perj:
                emit_av_norm(prev, h2, j)
            elif j == 3:
                emit_av_norm(prev, h2)

        def emit_av_norm(prev, h2, j=None):
            m, lt, ats, avs = prev
            h = 2 * m + h2
            avp = avs[h2]
            with tc.high_priority():
                r = rec_pool.tile([128, 4], F32, tag="rec", name="rec")
                if j is None:
                    # batched: one reciprocal + one strided tensor_tensor for
                    # all 4 j-destinations (r broadcast along e via stride-0)
                    nc.vector.reciprocal(r[:], avp[:, 64:260:65])
                    src = avp[:].rearrange("p (j e) -> p j e", j=4)[:, :, 0:64]
                    dst = vs3[:, 4 * lt : 4 * lt + 4, 64 * h : 64 * h + 64]
                    rb = r[:].rearrange("p (j o) -> p j o", j=4).broadcast_to(
                        (128, 4, 64))
                    nc.vector.tensor_tensor(dst, src, rb, mybir.AluOpType.mult)
                else:
                    nc.vector.reciprocal(
                        r[:, j : j + 1], avp[:, 65 * j + 64 : 65 * j + 65])
                    nc.vector.tensor_scalar_mul(
                        vs3[:, 4 * lt + j, 64 * h : 64 * h + 64],
                        avp[:, 65 * j : 65 * j + 64], r[:, j : j + 1])

        # out-projection pieces are split into an MM part and a DEFERRED
        # evacuation part (2 g-slots later): a piece's psum->sbuf copies used
        # to sit in the strict-FIFO ACT/DVE queues blocking 1-3us on the MM
        # chain while the PE stalled on the exps queued behind them.
        pending_ops = {}

        def emit_piece_mms(ls):
            ops = []
            for d2 in range(2):
                # alternate psum tags so the d2=1 chain does not wait for the
                # d2=0 evacuation (tr is free of transposes now)
                pool = out_ps if d2 == 0 else tr_ps
                op = pool.tile([128, 512], F32, tag=("outp" if d2 == 0 else "tr"),
                               name="outp")
                for p in range(4):
                    nc.tensor.matmul(
                        op[:], vt_t[ls][p][:], wo[p][:, 512 * d2 : 512 * d2 + 512],
                        start=(p == 0), stop=(p == 3))
                ops.append(op)
            pending_ops[ls] = ops

        def emit_piece_evac(ls):
            ops = pending_ops.pop(ls)
            osb = osb_pool.tile([128, D], BF16, tag="osb", name="osb")
            for d2 in range(2):
                dst = osb[:, 512 * d2 : 512 * d2 + 512]
                if d2 == 0:
                    nc.scalar.copy(dst, ops[d2][:])
                else:
                    nc.vector.tensor_copy(dst, ops[d2][:])
                nc.sync.dma_start(
                    out_d[128 * ls : 128 * ls + 128, 512 * d2 : 512 * d2 + 512],
                    dst)

        def emit_block(m, lt, prev, extras, perj=False):
            ats = {0: [], 1: []}
            for g in range(8):
                st0 = st_ps.tile([128, 1024], F32, tag="st0", name="st0")
                st1 = {c2: st_ps.tile([128, 512], F32, tag=f"st1{c2}", name=f"st1{c2}")
                       for c2 in range(2)}
                # interleave row groups (h=0, h=1, h=0, h=1): each MM's
                # LDWEIGHTS targets the opposite array half of the in-flight
                # matmul, so the per-subarray pull-ahead hides it (emitting
                # both c2 of one half back-to-back exposes ~107ns per pair)
                for c2 in range(2):
                    s = 2 * g + c2
                    nc.tensor.matmul(
                        st0[:, 512 * c2 : 512 * c2 + 512],
                        kT[m][0:64, 128 * s : 128 * s + 128],
                        qT[m][0:64, 512 * lt : 512 * lt + 512],
                        start=True, stop=True)
                    nc.tensor.matmul(
                        st1[c2][:],
                        kT[m][64:128, 128 * s : 128 * s + 128],
                        qT[m][64:128, 512 * lt : 512 * lt + 512],
                        start=True, stop=True)
                at0 = at_pool.tile([128, 1024], BF16, tag="at0", name="at0")
                nc.scalar.activation(at0[:], st0[:], AF.Exp, scale=SCALE)
                ats[0].append(at0)
                at1 = at_pool.tile([128, 1024], BF16, tag="at1", name="at1")
                for c2 in range(2):
                    dst = at1[:, 512 * c2 : 512 * c2 + 512]
                    # pair 3: ACT also carries the piece-evac copies, so all
                    # h1-halves go to the DVE Schraudolph path there
                    if m < 3 and (g, c2) in ACT_H1_HALVES:
                        nc.scalar.activation(dst, st1[c2][:], AF.Exp, scale=SCALE)
                    else:
                        nc.vector.tensor_scalar(
                            dst.bitcast(mybir.dt.int16), st1[c2][:],
                            EXP_A16, EXP_B16,
                            mybir.AluOpType.mult, mybir.AluOpType.add)
                ats[1].append(at1)
                if prev is not None:
                    emit_av_slice(prev, g, perj=perj)
                for fn in extras.get(g, ()):
                    fn()
            return ats

        # ---- prologue: only q(0,0) + k(0,0); the rest streams into blocks ----
        emit_qk_proj(0, "q", 0, "st0")
        emit_qk_proj(0, "k", 0, "st10")

        rot = ["outp", "tr"]
        vrot = ["av0", "av1", "outp", "tr"]

        prev = None
        pq_pair = []
        carry_evac = []
        for m in range(4):
            if m < 3:
                # pair m+1 projection quanta, ordered by score-consumption
                # deadline in block (m+1, 0): k first (all s needed), q n=0
                # early (lt=0), q n=1..3 late (lt>=1 blocks).
                nx = m + 1
                pq_pair = [(nx, "k", 0), (nx, "q", 0), (nx, "k", 1),
                           (nx, "k", 2), (nx, "k", 3), (nx, "q", 1),
                           (nx, "q", 2), (nx, "q", 3)]
            for lt in range(NL):
                extras = {}
                if m == 0 and lt == 0:
                    # stream remaining pair-0 k (needed at g=2n) and q, plus
                    # the 16 v-projection groups, into the first block;
                    # evacuations go one slot behind their chains
                    for i, n in enumerate((1, 2, 3)):
                        extras.setdefault(2 * i, []).append(
                            lambda n=n: emit_qk_mms(0, "k", n, rot[n % 2]))
                        extras.setdefault(2 * i + 1, []).append(
                            lambda n=n: emit_qk_evac(0, "k", n))
                        extras.setdefault(2 * i + 1, []).append(
                            lambda n=n: emit_qk_mms(0, "q", n, rot[(n + 1) % 2]))
                        extras.setdefault(2 * i + 2, []).append(
                            lambda n=n: emit_qk_evac(0, "q", n))
                    for s2 in range(NS):
                        extras.setdefault(s2 // 2, []).append(
                            lambda s2=s2: emit_v_mms(s2, vrot[s2 % 4]))
                        extras.setdefault(min(s2 // 2 + 1, 7), []).append(
                            lambda s2=s2: emit_v_evac(s2))
                elif m < 3 and lt >= 1:
                    # 3/3/2 projection quanta in blocks (m, 1..3)
                    for i in range(3 if lt < 3 else 2):
                        if pq_pair:
                            pm, w, n = pq_pair.pop(0)
                            extras.setdefault(2 * i + 2, []).append(
                                lambda pm=pm, w=w, n=n, i=i:
                                    emit_qk_mms(pm, w, n, rot[i % 2]))
                            extras.setdefault(2 * i + 3, []).append(
                                lambda pm=pm, w=w, n=n: emit_qk_evac(pm, w, n))
                perj = m == 3 and lt > 0
                if perj:
                    # prev AV (pair 3, per-j normalized) feeds the lt-1
                    # out-projection pieces: norm-j lands at slice g=2j+1 (so
                    # the p=3 transpose DMA goes there), piece j at g=2j+2,
                    # its evacuation deferred to g=2j+4 (or the next block).
                    for i, ls_c in enumerate(carry_evac):
                        extras.setdefault(2 * i, []).append(
                            lambda ls_c=ls_c: emit_piece_evac(ls_c))
                    del carry_evac[:]
                    for j in range(4):
                        ls = 4 * (lt - 1) + j
                        extras.setdefault(min(2 * j + 1, 7), []).append(
                            lambda ls=ls: emit_vt(ls, 3, nc.scalar))
                        extras.setdefault(min(2 * j + 2, 7), []).append(
                            lambda ls=ls: emit_piece_mms(ls))
                        if 2 * j + 4 <= 7:
                            extras.setdefault(2 * j + 4, []).append(
                                lambda ls=ls: emit_piece_evac(ls))
                        else:
                            carry_evac.append(ls)
                ats = emit_block(m, lt, prev, extras, perj=perj)
                if prev is not None and prev[0] < 3:
                    # prev block's batched norm just landed inside this block:
                    # launch its 4 V^T transpose DMAs (dep-ordered sync queue)
                    for j in range(4):
                        emit_vt(4 * prev[1] + j, prev[0])
                prev = (m, lt, ats, {})
        # final block's AV (per-j normalized) interleaved with the last
        # out-projection pieces so the tail pipeline stays short
        for g in range(8):
            if g in (0, 2) and carry_evac:
                emit_piece_evac(carry_evac.pop(0))
            emit_av_slice(prev, g, perj=True)
            if g % 2 == 1:
                emit_vt(12 + g // 2, 3, nc.scalar)
                emit_piece_mms(12 + g // 2)
            if g == 5:
                emit_piece_evac(12)
            if g == 7:
                emit_piece_evac(13)
        emit_piece_evac(14)
        emit_piece_evac(15)
        phase1_ctx.close()

    nc.compile()
    return nc


_NC_CACHE = []


def _make_in_maps(inputs):
    x = np.asarray(inputs["x"], dtype=np.float32)
    Wq = np.asarray(inputs["Wq"], dtype=np.float32)
    Wk = np.asarray(inputs["Wk"], dtype=np.float32)
    Wv = np.asarray(inputs["Wv"], dtype=np.float32)
    Wo = np.asarray(inputs["Wo"], dtype=np.float32)
    bq = np.asarray(inputs["bq"], dtype=np.float32)
    bf = ml_dtypes.bfloat16
    in_maps = []
    for c in range(N_CORES):
        b, hh = divmod(c, 2)
        sl = slice(DH * hh, DH * hh + DH)
        in_maps.append({
            "xT": np.ascontiguousarray(x[b].T).astype(bf),
            "wq": np.ascontiguousarray(Wq[:, sl]).astype(bf),
            "wk": np.ascontiguousarray(Wk[:, sl]).astype(bf),
            "wv": np.ascontiguousarray(Wv[:, sl]).astype(bf),
            "wo": np.ascontiguousarray(Wo[sl, :]).astype(bf),
            "bq": np.ascontiguousarray(bq[sl]).reshape(DH, 1).astype(np.float32),
        })
    return in_maps


def kernel(x, Wq, bq, Wk, bk, Wv, bv, Wo, bo):
    x = np.asarray(x, dtype=np.float32)
    Wq = np.asarray(Wq, dtype=np.float32)
    Wk = np.asarray(Wk, dtype=np.float32)
    Wv = np.asarray(Wv, dtype=np.float32)
    Wo = np.asarray(Wo, dtype=np.float32)
    bq = np.asarray(bq, dtype=np.float32)
    bv = np.asarray(bv, dtype=np.float32)
    bo = np.asarray(bo, dtype=np.float32)

    if not _NC_CACHE:
        _NC_CACHE.append(build_attention_nc())
    nc = _NC_CACHE[0]

    in_maps = _make_in_maps(dict(x=x, Wq=Wq, bq=bq, Wk=Wk, Wv=Wv, Wo=Wo))

    res = run_bass_kernel_spmd(nc, in_maps, list(range(N_CORES)))
    parts = [np.asarray(res.results[c]["out"]).astype(np.float32)
             for c in range(N_CORES)]
    out = np.stack([parts[2 * b] + parts[2 * b + 1] for b in range(B)])
    out += (bv @ Wo + bo)[None, None, :]
    return out.astype(np.float32)

